# revision 34
# baseline (speedup 1.0000x reference)
"""Trainium2 Bass kernel for varlen (ragged) BERT self-attention.

Strategy: tensor-parallel over heads. 16 heads across 8 NeuronCores ->
2 heads per core. Every core runs an IDENTICAL program (SPMD) on:
  - xt:   full hidden_states, host-padded so each sequence starts at a
          128-aligned token offset, transposed + bf16, laid out
          [128, n_ti, 8, 512] so each 512-token chunk is one contiguous
          8KB-per-partition DMA read.
  - wqk:  this core's q/k weight slices as matmul-lhsT blocks
          [8, 128, 256] bf16 (cols 0:128 = q, 128:256 = k).
  - wv:   this core's v weight slice as matmul-rhs blocks [8, 128, 128].
  - bias: q/k bias [128, 2] f32 (per-partition adds on eviction).
Output per core: raw [130, nnz_pad] f32: rows 0:64   = sum_k p*v (head0)
                                         row  64    = sum_k p   (head0)
                                         rows 65:129/129 same for head1.
Host divides by the denominator row, adds the v bias (algebraically
exact: softmax(s) @ (v+bv) = softmax(s)@v + bv), and gathers the
padded layout back to the ragged one.

On-chip per core:
  1. QKV projection, K=1024 in 8 chunks of 128:
     - q,k computed TRANSPOSED (stationary = W slices): qT/kT
       [128(=2h x 64d), nnz_pad] bf16, bias added during PSUM->SBUF
       eviction (DVE tensor_scalar add, per-partition).
     - v computed NATURAL (stationary = xt chunks, moving = Wv):
       v_nat [128(tok), chunk, 2, 65] bf16 with a preset ones column;
       no transposes needed anywhere.
  2. Attention per sequence (no packing; every sequence 128-aligned):
     scores[k,q] = kT.T @ qT per head with K=64 -> the two heads'
     matmuls auto-place on PE row tiles (0,0)/(64,0) and run
     CONCURRENTLY. exp on ACT over both heads in one instruction
     (scale=1/8 folded in). outT[65, q] accumulated as
     (v|ones).T @ exp with K<=128; the ones column gives the softmax
     denominator for free. PSUM -> SBUF copy (DVE) -> DMA out raw.

Emission interleaves per-jc attention into the QKV chunk stream
(chunks processed back-to-front, sequences emitted largest-first) so
the PE instruction stream stays dense end-to-end (HAM clock gate).
"""

import functools
import sys

import numpy as np

for _p in ("/opt/trn_rl_repo",):
    if _p not in sys.path:
        sys.path.append(_p)

import ml_dtypes  # noqa: E402

N_HEADS = 16
HEAD_DIM = 64
DIM = 1024
N_CORES = 8
HEADS_PER_CORE = N_HEADS // N_CORES  # 2


def _padded_units(lengths):
    """One unit per sequence at a 128-aligned padded offset."""
    units = []
    off = 0
    for L in lengths:
        if L == 0:
            continue
        units.append((off, L))
        off += ((L + 127) // 128) * 128
    nnz_pad = ((off + 511) // 512) * 512
    return tuple(units), nnz_pad


@functools.lru_cache(maxsize=4)
def _build(nnz_pad, units):
    """Build + compile the SPMD Bass program for the given ragged lengths."""
    import concourse.mybir as mybir
    import concourse.tile as tile
    from concourse import bacc

    f32 = mybir.dt.float32
    bf16 = mybir.dt.bfloat16
    Exp = mybir.ActivationFunctionType.Exp

    KC = DIM // 128  # 8 contraction chunks
    D = HEAD_DIM
    n_ti = nnz_pad // 512
    n_ch = nnz_pad // 128

    nc = bacc.Bacc("TRN2", target_bir_lowering=False, debug=False)
    xt = nc.declare_dram_parameter("xt", [128, n_ti, KC, 512], bf16, isOutput=False)
    wqk = nc.declare_dram_parameter("wqk", [KC, 128, 256], bf16, isOutput=False)
    wv = nc.declare_dram_parameter("wv", [KC, 128, 128], bf16, isOutput=False)
    bias = nc.declare_dram_parameter("bias", [128, 2], f32, isOutput=False)
    out = nc.declare_dram_parameter("out", [130, nnz_pad], f32, isOutput=True)

    with tile.TileContext(nc) as tc:
        with (
            tc.tile_pool(name="res", bufs=1) as res,
            tc.tile_pool(name="xp", bufs=4) as xp,
            tc.tile_pool(name="esp", bufs=26) as esp,
            tc.tile_pool(name="otp", bufs=4) as otp,
            tc.tile_pool(name="ps", bufs=1, space="PSUM") as ps,
        ):
            # --- resident tensors; startup DMAs split per-chunk and spread
            # across engine queues so the first matmul starts ASAP ---
            wqk_sb = res.tile([128, KC, 256], bf16)
            wv_sb = res.tile([128, KC, 128], bf16)
            bias_sb = res.tile([128, 2], f32)
            qT = res.tile([128, nnz_pad], bf16)
            kT = res.tile([128, nnz_pad], bf16)
            # v in natural layout: [tok(128), chunk, head, 64 v + 1 ones]
            v_nat = res.tile([128, n_ch, 2, D + 1], bf16)
            # HAM warm-up: dummy matmuls on a memset tile during the initial
            # DMA wait release the PE clock gate before real work arrives
            wrm = res.tile([128, 512], bf16)
            nc.vector.memset(wrm[:, :], 0.0)
            nc.gpsimd.memset(v_nat[:, :, :, D : D + 1], 1.0)
            xt_tiles = {}

            def ensure_xt(ti, split=False, eng=None):
                if ti >= n_ti or ti < 0 or ti in xt_tiles:
                    return
                t = xp.tile([128, KC, 512], bf16, tag="xt", name="xt_t")
                if split:
                    for kc in range(KC):
                        e = (nc.gpsimd if kc < 4 else nc.scalar) if eng is None else eng
                        e.dma_start(t[:, kc, :], xt[:, ti, kc, :])
                else:
                    (eng or nc.sync).dma_start(t[:], xt[:, ti, :, :])
                xt_tiles[ti] = t

            for kc in range(KC):
                nc.sync.dma_start(
                    wqk_sb[:, kc, :], wqk[kc, :, :]
                )
            ensure_xt(n_ti - 1, split=True)
            nc.scalar.dma_start(bias_sb[:], bias[:, :])
            nc.scalar.dma_start(
                wv_sb[:], wv[:, :, :].rearrange("a p m -> p a m")
            )
            for _ in range(7):
                dm = ps.tile([128, 512], f32, tag="mm", bufs=2, name="dm")
                nc.tensor.matmul(
                    dm[:, :], wrm[:, 0:128], wrm[:, :], start=True, stop=True
                )
            ensure_xt(n_ti - 2)

            # --- QKV feeder: yields 3 groups per token chunk (q, k+v0, v1)
            # so attention emission can interleave dense PE work ---
            state = {"ti_next": n_ti}

            def _qkv_groups():
                for ti in range(n_ti - 1, -1, -1):
                    t0 = ti * 512
                    ensure_xt(ti)
                    ensure_xt(ti - 1)
                    ensure_xt(ti - 2)
                    xt_tile = xt_tiles.pop(ti)
                    # q group: stationary W, moving xt
                    mmq = ps.tile([128, 512], f32, tag="mm", bufs=2, name="mmq")
                    for kc in range(KC):
                        nc.tensor.matmul(
                            mmq[:, :],
                            wqk_sb[:, kc, 0:128],
                            xt_tile[:, kc, :],
                            start=(kc == 0),
                            stop=(kc == KC - 1),
                        )
                    nc.vector.tensor_scalar_add(
                        qT[:, t0 : t0 + 512], mmq[:, :], bias_sb[:, 0:1]
                    )
                    yield
                    # k group interleaved with first half of v (v: stationary
                    # xt chunk, moving Wv -> LDW-heavy; hide under k streams)
                    mmk = ps.tile([128, 512], f32, tag="mm", bufs=2, name="mmk")
                    mmv = ps.tile([128, 4, 2, D], f32, tag="mm", bufs=2, name="mmv")
                    vs = [(c, kc) for c in range(4) for kc in range(KC)]
                    vi = 0

                    def _vmm(c, kc):
                        nc.tensor.matmul(
                            mmv[:, c, :, :],
                            xt_tile[:, kc, c * 128 : (c + 1) * 128],
                            wv_sb[:, kc, :],
                            start=(kc == 0),
                            stop=(kc == KC - 1),
                        )

                    for kc in range(KC):
                        nc.tensor.matmul(
                            mmk[:, :],
                            wqk_sb[:, kc, 128:256],
                            xt_tile[:, kc, :],
                            start=(kc == 0),
                            stop=(kc == KC - 1),
                        )
                        for _ in range(2):
                            _vmm(*vs[vi])
                            vi += 1
                    nc.vector.tensor_scalar_add(
                        kT[:, t0 : t0 + 512], mmk[:, :], bias_sb[:, 1:2]
                    )
                    yield
                    # rest of v
                    while vi < len(vs):
                        _vmm(*vs[vi])
                        vi += 1
                    nc.vector.tensor_copy(
                        v_nat[:, ti * 4 : ti * 4 + 4, :, 0:D], mmv[:, :, :, :]
                    )
                    state["ti_next"] = ti
                    yield

            feeder = _qkv_groups()

            # pacing: spread remaining feeder groups over remaining
            # attention jc-iterations (recomputed each step)
            n_groups = 3 * n_ti
            n_iters = sum(
                ((L + 127) // 128) * ((L + 511) // 512) for _, L in units
            )
            pace = {"acc": 0.0, "groups": n_groups, "iters": n_iters}

            def feed(n):
                for _ in range(n):
                    if next(feeder, "done") == "done":
                        break
                    pace["groups"] -= 1

            def feed_cb():
                if pace["iters"] > 0:
                    pace["acc"] += pace["groups"] / pace["iters"]
                pace["iters"] -= 1
                k = min(int(pace["acc"]), pace["groups"])
                if k > 0:
                    pace["acc"] -= k
                    feed(k)
                else:
                    # no feeder work this slot: cheap PE keepalive so the
                    # HAM clock gate stays released through ACT-bound slots
                    dm = ps.tile([128, 64], f32, tag="mm", bufs=2, name="dm")
                    nc.tensor.matmul(
                        dm[:, :], wrm[:, 0:128], wrm[:, 0:64],
                        start=True, stop=True,
                    )

            # --- deferred out-matmul machinery -------------------------------
            # Out matmuls run as 64-row-mode split-K pairs (tokens 0:64 on PE
            # row tile (0,0), tokens 64:128 on (64,0)) so the whole attention
            # stream stays in one PE tiling mode (mode switches cost ~300ns).
            # Each (qc, head) package accumulates into ov[:, 0/1, :] (a/b
            # banks); the a+b merge happens in the DVE eviction for free.
            # Packages are queued and drained during the FOLLOWING qc's score
            # slots, overlapping the ACT-paced exp stream.
            outq = []  # pending out-pairs: (emit_fn,)

            def drain_out(k):
                while k > 0 and outq:
                    outq.pop(0)()
                    k -= 1

            def push_pkg(entries, dmas):
                """Queue out-matmuls + eviction for one package; both heads.

                entries: [(gb, jc_global_chunk, es, nj, off, nq_u)] — each
                contributes its unit's k-chunk into packed ov columns
                [off, off+nq_u). dmas: [(q0_hbm, off, nq_u)].
                width: total packed columns (<= 512).
                """
                width = max(e[4] + e[5] for e in entries)
                bids = [i for i, e in enumerate(entries) if e[3] > 64]
                # column ranges whose b-half bank actually gets written --
                # only those may be merged in (elsewhere the bank is stale)
                bcols = sorted({(e[4], e[5]) for e in entries if e[3] > 64})
                merged = []
                for off_u, n_u in bcols:
                    if merged and merged[-1][0] + merged[-1][1] == off_u:
                        merged[-1] = (merged[-1][0], merged[-1][1] + n_u)
                    else:
                        merged.append((off_u, n_u))
                for h in range(2):
                    ov = ps.tile(
                        [D + 1, 2, 512], f32, tag="ov", bufs=1, name="ov"
                    )

                    def mk(i, e, h=h, ov=ov):
                        gc, es, nj, off, nq_u = e[1], e[2], e[3], e[4], e[5]

                        def go():
                            na = min(nj, 64)
                            nc.tensor.matmul(
                                ov[:, 0, off : off + nq_u],
                                v_nat[0:na, gc, h, :],
                                es[0:na, h, :nq_u],
                                start=(i == 0),
                                stop=(i == len(entries) - 1),
                            )
                            if nj > 64:
                                nc.tensor.matmul(
                                    ov[:, 1, off : off + nq_u],
                                    v_nat[64:nj, gc, h, :],
                                    es[64:nj, h, :nq_u],
                                    start=(i == bids[0]),
                                    stop=(i == bids[-1]),
                                )

                        return go

                    for i, e in enumerate(entries):
                        outq.append(mk(i, e))

                    def evict(h=h, ov=ov, oc=dmas):
                        ot = otp.tile(
                            [D + 1, 2, 512], f32, tag="ot", name=f"ot{h}"
                        )
                        nc.vector.tensor_copy(
                            ot[:, h, :width], ov[:, 0, :width]
                        )
                        for off_u, n_u in merged:
                            nc.vector.tensor_add(
                                ot[:, h, off_u : off_u + n_u],
                                ot[:, h, off_u : off_u + n_u],
                                ov[:, 1, off_u : off_u + n_u],
                            )
                        for q0, off, nq_u in oc:
                            nc.sync.dma_start(
                                out[
                                    65 * h : 65 * h + 65, q0 : q0 + nq_u
                                ],
                                ot[:, h, off : off + nq_u],
                            )

                    outq.append(evict)

            def emit_scores(O, L, q0, nq, jcs):
                """Score+exp slots for one unit's (qc); returns es entries."""
                gb = O // 128
                nk = (L + 127) // 128
                out_entries = []
                for jc in jcs:
                    feed_cb()
                    nj = min(128, L - jc * 128)
                    k0 = O + jc * 128
                    sps = ps.tile(
                        [128, 2, 512], f32, tag="sc", bufs=2, name="sps"
                    )
                    # two heads on PE row tiles (0,0) / (64,0): concurrent
                    for h in range(2):
                        p0 = D * h
                        nc.tensor.matmul(
                            sps[:nj, h, :nq],
                            kT[p0 : p0 + D, k0 : k0 + nj],
                            qT[p0 : p0 + D, q0 : q0 + nq],
                            start=True,
                            stop=True,
                        )
                    es = esp.tile([128, 2, 512], bf16, tag="es", name="es")
                    nc.scalar.activation(
                        es[:nj, :, :nq], sps[:nj, :, :nq], Exp, scale=0.125
                    )
                    out_entries.append((gb, gb + jc, es, nj))
                    # keep the out-queue drained by the time slots run out
                    k = 3
                    if 0 < pace["iters"] < 40:
                        k = min(6, max(3, -(-len(outq) // pace["iters"])))
                    drain_out(k)
                return out_entries

            def emit_attention(O, L):
                nk = (L + 127) // 128
                for qc in range((L + 511) // 512):
                    q0 = O + qc * 512
                    nq = min(512, L - qc * 512)
                    ents = emit_scores(O, L, q0, nq, range(nk))
                    push_pkg(
                        [(gb, gc, es, nj, 0, nq) for gb, gc, es, nj in ents],
                        [(q0, 0, nq)],
                    )

            def emit_attention_group(grp):
                """Several small units (each L<=512, total<=512) share one
                package: their out columns pack side by side in one ov."""
                entries, dmas = [], []
                off = 0
                for O, L in grp:
                    nk = (L + 127) // 128
                    ents = emit_scores(O, L, O, L, range(nk))
                    entries.extend(
                        (gb, gc, es, nj, off, L) for gb, gc, es, nj in ents
                    )
                    dmas.append((O, off, L))
                    off += L
                push_pkg(entries, dmas)

            # --- interleaved emission: chunks back-to-front via the feeder;
            # a unit is ready once all chunks covering [O, O+L) are emitted.
            pending = sorted(units, key=lambda u: u[0], reverse=True)
            i = 0
            while i < len(pending):
                O, L = pending[i]
                if L >= 512:
                    while state["ti_next"] * 512 > O:
                        feed(1)
                    emit_attention(O, L)
                    i += 1
                else:
                    grp, tot = [], 0
                    while (
                        i < len(pending)
                        and pending[i][1] < 512
                        and tot + pending[i][1] <= 512
                    ):
                        grp.append(pending[i])
                        tot += pending[i][1]
                        i += 1
                    while state["ti_next"] * 512 > grp[-1][0]:
                        feed(1)
                    emit_attention_group(grp)
            while outq:
                outq.pop(0)()
                feed_cb()  # keepalive through the final drain
            feed(n_groups)  # drain any leftovers

    nc.compile()
    return nc


def _prepare(hidden_states, Wqkv_weight, Wqkv_bias, cu_seqlens):
    """Host-side sharding prep. Returns (nc, in_maps, meta)."""
    hs = np.asarray(hidden_states, dtype=np.float32)
    W = np.asarray(Wqkv_weight, dtype=np.float32)
    b = np.asarray(Wqkv_bias, dtype=np.float32).reshape(-1)
    cs = np.asarray(cu_seqlens).astype(np.int64).reshape(-1)
    nnz, dim = hs.shape
    assert dim == DIM and W.shape == (3 * DIM, DIM)
    lengths = tuple(int(cs[i + 1] - cs[i]) for i in range(len(cs) - 1))
    assert sum(lengths) == nnz, (lengths, nnz)

    units, nnz_pad = _padded_units(lengths)
    nc = _build(nnz_pad, units)

    # padded token index map: real token t -> padded column pad_idx[t]
    pad_idx = np.empty(nnz, dtype=np.int64)
    t = 0
    for (O, L) in units:
        pad_idx[t : t + L] = np.arange(O, O + L)
        t += L

    xt_pad = np.zeros((DIM, nnz_pad), dtype=np.float32)
    xt_pad[:, pad_idx] = hs.T
    n_ti = nnz_pad // 512
    xt_np = np.ascontiguousarray(
        xt_pad.reshape(DIM // 128, 128, n_ti, 512).transpose(1, 2, 0, 3)
    ).astype(ml_dtypes.bfloat16)

    in_maps = []
    for c in range(N_CORES):
        r0 = c * HEADS_PER_CORE * HEAD_DIM  # 128c
        Wq = W[r0 : r0 + 128, :]
        Wk = W[DIM + r0 : DIM + r0 + 128, :]
        Wv = W[2 * DIM + r0 : 2 * DIM + r0 + 128, :]
        wqk_np = np.ascontiguousarray(
            np.concatenate([Wq.T, Wk.T], axis=1).reshape(DIM // 128, 128, 256)
        ).astype(ml_dtypes.bfloat16)
        wv_np = np.ascontiguousarray(
            Wv.T.reshape(DIM // 128, 128, 128)
        ).astype(ml_dtypes.bfloat16)
        bias_np = np.ascontiguousarray(
            np.stack([b[r0 : r0 + 128], b[DIM + r0 : DIM + r0 + 128]], axis=1)
        )
        in_maps.append(
            {"xt": xt_np, "wqk": wqk_np, "wv": wv_np, "bias": bias_np}
        )
    meta = {"pad_idx": pad_idx, "nnz": nnz, "bv": b[2 * DIM :]}
    return nc, in_maps, meta


def _postprocess(raws, meta):
    """raws: list of per-core [130, nnz_pad] f32 -> full (nnz, 1024) f32."""
    pad_idx = meta["pad_idx"]
    nnz = meta["nnz"]
    bv = meta["bv"]
    out = np.empty((nnz, DIM), dtype=np.float32)
    for c in range(N_CORES):
        raw = np.asarray(raws[c], dtype=np.float32)
        for h in range(HEADS_PER_CORE):
            num = raw[65 * h : 65 * h + 64][:, pad_idx]  # (64, nnz)
            den = raw[65 * h + 64][pad_idx]  # (nnz,)
            col = c * 128 + h * 64
            out[:, col : col + 64] = num.T / den[:, None] + bv[col : col + 64]
    return out


def kernel(hidden_states, Wqkv_weight, Wqkv_bias, cu_seqlens, max_seqlen=None):
    from concourse.bass_utils import run_bass_kernel_spmd

    nc, in_maps, meta = _prepare(
        hidden_states, Wqkv_weight, Wqkv_bias, cu_seqlens
    )
    res = run_bass_kernel_spmd(nc, in_maps, list(range(N_CORES)))
    return _postprocess(
        [res.results[c]["out"] for c in range(N_CORES)], meta
    )


# revision 35
# speedup vs baseline: 1.0017x; 1.0017x over previous
"""Trainium2 Bass kernel for varlen (ragged) BERT self-attention.

Strategy: tensor-parallel over heads. 16 heads across 8 NeuronCores ->
2 heads per core. Every core runs an IDENTICAL program (SPMD) on:
  - xt:   full hidden_states, host-padded so each sequence starts at a
          128-aligned token offset, transposed + bf16, laid out
          [128, n_ti, 8, 512] so each 512-token chunk is one contiguous
          8KB-per-partition DMA read.
  - wqk:  this core's q/k weight slices as matmul-lhsT blocks
          [8, 128, 256] bf16 (cols 0:128 = q, 128:256 = k).
  - wv:   this core's v weight slice as matmul-rhs blocks [8, 128, 128].
  - bias: q/k bias [128, 2] f32 (per-partition adds on eviction).
Output per core: raw [130, nnz_pad] f32: rows 0:64   = sum_k p*v (head0)
                                         row  64    = sum_k p   (head0)
                                         rows 65:129/129 same for head1.
Host divides by the denominator row, adds the v bias (algebraically
exact: softmax(s) @ (v+bv) = softmax(s)@v + bv), and gathers the
padded layout back to the ragged one.

On-chip per core:
  1. QKV projection, K=1024 in 8 chunks of 128:
     - q,k computed TRANSPOSED (stationary = W slices): qT/kT
       [128(=2h x 64d), nnz_pad] bf16, bias added during PSUM->SBUF
       eviction (DVE tensor_scalar add, per-partition).
     - v computed NATURAL (stationary = xt chunks, moving = Wv):
       v_nat [128(tok), chunk, 2, 65] bf16 with a preset ones column;
       no transposes needed anywhere.
  2. Attention per sequence (no packing; every sequence 128-aligned):
     scores[k,q] = kT.T @ qT per head with K=64 -> the two heads'
     matmuls auto-place on PE row tiles (0,0)/(64,0) and run
     CONCURRENTLY. exp on ACT over both heads in one instruction
     (scale=1/8 folded in). outT[65, q] accumulated as
     (v|ones).T @ exp with K<=128; the ones column gives the softmax
     denominator for free. PSUM -> SBUF copy (DVE) -> DMA out raw.

Emission interleaves per-jc attention into the QKV chunk stream
(chunks processed back-to-front, sequences emitted largest-first) so
the PE instruction stream stays dense end-to-end (HAM clock gate).
"""

import functools
import sys

import numpy as np

for _p in ("/opt/trn_rl_repo",):
    if _p not in sys.path:
        sys.path.append(_p)

import ml_dtypes  # noqa: E402

N_HEADS = 16
HEAD_DIM = 64
DIM = 1024
N_CORES = 8
HEADS_PER_CORE = N_HEADS // N_CORES  # 2


def _padded_units(lengths):
    """One unit per sequence at a 128-aligned padded offset."""
    units = []
    off = 0
    for L in lengths:
        if L == 0:
            continue
        units.append((off, L))
        off += ((L + 127) // 128) * 128
    nnz_pad = ((off + 511) // 512) * 512
    return tuple(units), nnz_pad


@functools.lru_cache(maxsize=4)
def _build(nnz_pad, units):
    """Build + compile the SPMD Bass program for the given ragged lengths."""
    import concourse.mybir as mybir
    import concourse.tile as tile
    from concourse import bacc

    f32 = mybir.dt.float32
    bf16 = mybir.dt.bfloat16
    Exp = mybir.ActivationFunctionType.Exp

    KC = DIM // 128  # 8 contraction chunks
    D = HEAD_DIM
    n_ti = nnz_pad // 512
    n_ch = nnz_pad // 128

    nc = bacc.Bacc("TRN2", target_bir_lowering=False, debug=False)
    xt = nc.declare_dram_parameter("xt", [128, n_ti, KC, 512], bf16, isOutput=False)
    wqk = nc.declare_dram_parameter("wqk", [KC, 128, 256], bf16, isOutput=False)
    wv = nc.declare_dram_parameter("wv", [KC, 128, 128], bf16, isOutput=False)
    bias = nc.declare_dram_parameter("bias", [128, 2], f32, isOutput=False)
    out = nc.declare_dram_parameter("out", [130, nnz_pad], f32, isOutput=True)

    with tile.TileContext(nc) as tc:
        with (
            tc.tile_pool(name="res", bufs=1) as res,
            tc.tile_pool(name="xp", bufs=4) as xp,
            tc.tile_pool(name="esp", bufs=26) as esp,
            tc.tile_pool(name="otp", bufs=4) as otp,
            tc.tile_pool(name="ps", bufs=1, space="PSUM") as ps,
        ):
            # --- resident tensors; startup DMAs split per-chunk and spread
            # across engine queues so the first matmul starts ASAP ---
            wqk_sb = res.tile([128, KC, 256], bf16)
            wv_sb = res.tile([128, KC, 128], bf16)
            bias_sb = res.tile([128, 2], f32)
            qT = res.tile([128, nnz_pad], bf16)
            kT = res.tile([128, nnz_pad], bf16)
            # v in natural layout: [tok(128), chunk, head, 64 v + 1 ones]
            v_nat = res.tile([128, n_ch, 2, D + 1], bf16)
            # HAM warm-up: dummy matmuls on a memset tile during the initial
            # DMA wait release the PE clock gate before real work arrives
            wrm = res.tile([128, 512], bf16)
            nc.vector.memset(wrm[:, :], 0.0)
            nc.gpsimd.memset(v_nat[:, :, :, D : D + 1], 1.0)
            xt_tiles = {}

            def ensure_xt(ti, split=False, eng=None):
                if ti >= n_ti or ti < 0 or ti in xt_tiles:
                    return
                t = xp.tile([128, KC, 512], bf16, tag="xt", name="xt_t")
                if split:
                    for kc in range(KC):
                        e = (nc.gpsimd if kc < 4 else nc.scalar) if eng is None else eng
                        e.dma_start(t[:, kc, :], xt[:, ti, kc, :])
                else:
                    (eng or nc.sync).dma_start(t[:], xt[:, ti, :, :])
                xt_tiles[ti] = t

            for kc in range(KC):
                nc.sync.dma_start(
                    wqk_sb[:, kc, :], wqk[kc, :, :]
                )
            ensure_xt(n_ti - 1, split=True)
            nc.scalar.dma_start(bias_sb[:], bias[:, :])
            nc.scalar.dma_start(
                wv_sb[:], wv[:, :, :].rearrange("a p m -> p a m")
            )
            for _ in range(7):
                dm = ps.tile([128, 512], f32, tag="mm", bufs=2, name="dm")
                nc.tensor.matmul(
                    dm[:, :], wrm[:, 0:128], wrm[:, :], start=True, stop=True
                )
            ensure_xt(n_ti - 2)

            # --- QKV feeder: yields 3 groups per token chunk (q, k+v0, v1)
            # so attention emission can interleave dense PE work ---
            state = {"ti_next": n_ti}

            def _qkv_groups():
                for ti in range(n_ti - 1, -1, -1):
                    t0 = ti * 512
                    ensure_xt(ti)
                    ensure_xt(ti - 1)
                    ensure_xt(ti - 2)
                    xt_tile = xt_tiles.pop(ti)
                    # q group: stationary W, moving xt
                    mmq = ps.tile([128, 512], f32, tag="mm", bufs=2, name="mmq")
                    for kc in range(KC):
                        nc.tensor.matmul(
                            mmq[:, :],
                            wqk_sb[:, kc, 0:128],
                            xt_tile[:, kc, :],
                            start=(kc == 0),
                            stop=(kc == KC - 1),
                        )
                    nc.vector.tensor_scalar_add(
                        qT[:, t0 : t0 + 512], mmq[:, :], bias_sb[:, 0:1]
                    )
                    yield
                    # k group interleaved with first half of v (v: stationary
                    # xt chunk, moving Wv -> LDW-heavy; hide under k streams)
                    mmk = ps.tile([128, 512], f32, tag="mm", bufs=2, name="mmk")
                    mmv = ps.tile([128, 4, 2, D], f32, tag="mm", bufs=2, name="mmv")
                    vs = [(c, kc) for c in range(4) for kc in range(KC)]
                    vi = 0

                    def _vmm(c, kc):
                        nc.tensor.matmul(
                            mmv[:, c, :, :],
                            xt_tile[:, kc, c * 128 : (c + 1) * 128],
                            wv_sb[:, kc, :],
                            start=(kc == 0),
                            stop=(kc == KC - 1),
                        )

                    for kc in range(KC):
                        nc.tensor.matmul(
                            mmk[:, :],
                            wqk_sb[:, kc, 128:256],
                            xt_tile[:, kc, :],
                            start=(kc == 0),
                            stop=(kc == KC - 1),
                        )
                        for _ in range(2):
                            _vmm(*vs[vi])
                            vi += 1
                    nc.vector.tensor_scalar_add(
                        kT[:, t0 : t0 + 512], mmk[:, :], bias_sb[:, 1:2]
                    )
                    yield
                    # rest of v
                    while vi < len(vs):
                        _vmm(*vs[vi])
                        vi += 1
                    nc.vector.tensor_copy(
                        v_nat[:, ti * 4 : ti * 4 + 4, :, 0:D], mmv[:, :, :, :]
                    )
                    state["ti_next"] = ti
                    yield

            feeder = _qkv_groups()

            # pacing: spread remaining feeder groups over remaining
            # attention jc-iterations (recomputed each step)
            n_groups = 3 * n_ti
            n_iters = sum(
                ((L + 127) // 128) * ((L + 511) // 512) for _, L in units
            )
            pace = {"acc": 0.0, "groups": n_groups, "iters": n_iters}

            def feed(n):
                for _ in range(n):
                    if next(feeder, "done") == "done":
                        break
                    pace["groups"] -= 1

            def feed_cb():
                if pace["iters"] > 0:
                    pace["acc"] += pace["groups"] / pace["iters"]
                pace["iters"] -= 1
                k = min(int(pace["acc"]), pace["groups"])
                if k >= 2 or k == pace["groups"] > 0:
                    # pull feeder groups in pairs: halves the number of
                    # 64<->128 PE tiling-mode switch boundaries
                    pace["acc"] -= k
                    feed(k)
                else:
                    # no feeder work this slot: cheap PE keepalive so the
                    # HAM clock gate stays released through ACT-bound slots.
                    # K=64, M=128 keeps it in the attention tiling mode
                    # (no mode-switch drain on either side).
                    dm = ps.tile([128, 64], f32, tag="mm", bufs=2, name="dm")
                    nc.tensor.matmul(
                        dm[:, :], wrm[0:64, 0:128], wrm[0:64, 0:64],
                        start=True, stop=True,
                    )

            # --- deferred out-matmul machinery -------------------------------
            # Out matmuls run as 64-row-mode split-K pairs (tokens 0:64 on PE
            # row tile (0,0), tokens 64:128 on (64,0)) so the whole attention
            # stream stays in one PE tiling mode (mode switches cost ~300ns).
            # Each (qc, head) package accumulates into ov[:, 0/1, :] (a/b
            # banks); the a+b merge happens in the DVE eviction for free.
            # Packages are queued and drained during the FOLLOWING qc's score
            # slots, overlapping the ACT-paced exp stream.
            outq = []  # pending out-pairs: (emit_fn,)

            def drain_out(k):
                while k > 0 and outq:
                    outq.pop(0)()
                    k -= 1

            def push_pkg(entries, dmas):
                """Queue out-matmuls + eviction for one package; both heads.

                entries: [(gb, jc_global_chunk, es, nj, off, nq_u)] — each
                contributes its unit's k-chunk into packed ov columns
                [off, off+nq_u). dmas: [(q0_hbm, off, nq_u)].
                width: total packed columns (<= 512).
                """
                width = max(e[4] + e[5] for e in entries)
                bids = [i for i, e in enumerate(entries) if e[3] > 64]
                # column ranges whose b-half bank actually gets written --
                # only those may be merged in (elsewhere the bank is stale)
                bcols = sorted({(e[4], e[5]) for e in entries if e[3] > 64})
                merged = []
                for off_u, n_u in bcols:
                    if merged and merged[-1][0] + merged[-1][1] == off_u:
                        merged[-1] = (merged[-1][0], merged[-1][1] + n_u)
                    else:
                        merged.append((off_u, n_u))
                for h in range(2):
                    ov = ps.tile(
                        [D + 1, 2, 512], f32, tag="ov", bufs=1, name="ov"
                    )

                    def mk(i, e, h=h, ov=ov):
                        gc, es, nj, off, nq_u = e[1], e[2], e[3], e[4], e[5]

                        def go():
                            na = min(nj, 64)
                            nc.tensor.matmul(
                                ov[:, 0, off : off + nq_u],
                                v_nat[0:na, gc, h, :],
                                es[0:na, h, :nq_u],
                                start=(i == 0),
                                stop=(i == len(entries) - 1),
                            )
                            if nj > 64:
                                nc.tensor.matmul(
                                    ov[:, 1, off : off + nq_u],
                                    v_nat[64:nj, gc, h, :],
                                    es[64:nj, h, :nq_u],
                                    start=(i == bids[0]),
                                    stop=(i == bids[-1]),
                                )

                        return go

                    for i, e in enumerate(entries):
                        outq.append(mk(i, e))

                    def evict(h=h, ov=ov, oc=dmas):
                        ot = otp.tile(
                            [D + 1, 2, 512], f32, tag="ot", name=f"ot{h}"
                        )
                        nc.vector.tensor_copy(
                            ot[:, h, :width], ov[:, 0, :width]
                        )
                        for off_u, n_u in merged:
                            nc.vector.tensor_add(
                                ot[:, h, off_u : off_u + n_u],
                                ot[:, h, off_u : off_u + n_u],
                                ov[:, 1, off_u : off_u + n_u],
                            )
                        for q0, off, nq_u in oc:
                            nc.sync.dma_start(
                                out[
                                    65 * h : 65 * h + 65, q0 : q0 + nq_u
                                ],
                                ot[:, h, off : off + nq_u],
                            )

                    outq.append(evict)

            def emit_scores(O, L, q0, nq, jcs):
                """Score+exp slots for one unit's (qc); returns es entries."""
                gb = O // 128
                nk = (L + 127) // 128
                out_entries = []
                for jc in jcs:
                    feed_cb()
                    nj = min(128, L - jc * 128)
                    k0 = O + jc * 128
                    sps = ps.tile(
                        [128, 2, 512], f32, tag="sc", bufs=2, name="sps"
                    )
                    # two heads on PE row tiles (0,0) / (64,0): concurrent
                    for h in range(2):
                        p0 = D * h
                        nc.tensor.matmul(
                            sps[:nj, h, :nq],
                            kT[p0 : p0 + D, k0 : k0 + nj],
                            qT[p0 : p0 + D, q0 : q0 + nq],
                            start=True,
                            stop=True,
                        )
                    es = esp.tile([128, 2, 512], bf16, tag="es", name="es")
                    nc.scalar.activation(
                        es[:nj, :, :nq], sps[:nj, :, :nq], Exp, scale=0.125
                    )
                    out_entries.append((gb, gb + jc, es, nj))
                    # keep the out-queue drained by the time slots run out
                    k = 3
                    if 0 < pace["iters"] < 40:
                        k = min(6, max(3, -(-len(outq) // pace["iters"])))
                    drain_out(k)
                return out_entries

            def emit_attention(O, L):
                nk = (L + 127) // 128
                for qc in range((L + 511) // 512):
                    q0 = O + qc * 512
                    nq = min(512, L - qc * 512)
                    ents = emit_scores(O, L, q0, nq, range(nk))
                    push_pkg(
                        [(gb, gc, es, nj, 0, nq) for gb, gc, es, nj in ents],
                        [(q0, 0, nq)],
                    )

            def emit_attention_group(grp):
                """Several small units (each L<=512, total<=512) share one
                package: their out columns pack side by side in one ov."""
                entries, dmas = [], []
                off = 0
                for O, L in grp:
                    nk = (L + 127) // 128
                    ents = emit_scores(O, L, O, L, range(nk))
                    entries.extend(
                        (gb, gc, es, nj, off, L) for gb, gc, es, nj in ents
                    )
                    dmas.append((O, off, L))
                    off += L
                push_pkg(entries, dmas)

            # --- interleaved emission: chunks back-to-front via the feeder;
            # a unit is ready once all chunks covering [O, O+L) are emitted.
            pending = sorted(units, key=lambda u: u[0], reverse=True)
            i = 0
            while i < len(pending):
                O, L = pending[i]
                if L >= 512:
                    while state["ti_next"] * 512 > O:
                        feed(1)
                    emit_attention(O, L)
                    i += 1
                else:
                    grp, tot = [], 0
                    while (
                        i < len(pending)
                        and pending[i][1] < 512
                        and tot + pending[i][1] <= 512
                    ):
                        grp.append(pending[i])
                        tot += pending[i][1]
                        i += 1
                    while state["ti_next"] * 512 > grp[-1][0]:
                        feed(1)
                    emit_attention_group(grp)
            while outq:
                outq.pop(0)()
                feed_cb()  # keepalive through the final drain
            feed(n_groups)  # drain any leftovers

    nc.compile()
    return nc


def _prepare(hidden_states, Wqkv_weight, Wqkv_bias, cu_seqlens):
    """Host-side sharding prep. Returns (nc, in_maps, meta)."""
    hs = np.asarray(hidden_states, dtype=np.float32)
    W = np.asarray(Wqkv_weight, dtype=np.float32)
    b = np.asarray(Wqkv_bias, dtype=np.float32).reshape(-1)
    cs = np.asarray(cu_seqlens).astype(np.int64).reshape(-1)
    nnz, dim = hs.shape
    assert dim == DIM and W.shape == (3 * DIM, DIM)
    lengths = tuple(int(cs[i + 1] - cs[i]) for i in range(len(cs) - 1))
    assert sum(lengths) == nnz, (lengths, nnz)

    units, nnz_pad = _padded_units(lengths)
    nc = _build(nnz_pad, units)

    # padded token index map: real token t -> padded column pad_idx[t]
    pad_idx = np.empty(nnz, dtype=np.int64)
    t = 0
    for (O, L) in units:
        pad_idx[t : t + L] = np.arange(O, O + L)
        t += L

    xt_pad = np.zeros((DIM, nnz_pad), dtype=np.float32)
    xt_pad[:, pad_idx] = hs.T
    n_ti = nnz_pad // 512
    xt_np = np.ascontiguousarray(
        xt_pad.reshape(DIM // 128, 128, n_ti, 512).transpose(1, 2, 0, 3)
    ).astype(ml_dtypes.bfloat16)

    in_maps = []
    for c in range(N_CORES):
        r0 = c * HEADS_PER_CORE * HEAD_DIM  # 128c
        Wq = W[r0 : r0 + 128, :]
        Wk = W[DIM + r0 : DIM + r0 + 128, :]
        Wv = W[2 * DIM + r0 : 2 * DIM + r0 + 128, :]
        wqk_np = np.ascontiguousarray(
            np.concatenate([Wq.T, Wk.T], axis=1).reshape(DIM // 128, 128, 256)
        ).astype(ml_dtypes.bfloat16)
        wv_np = np.ascontiguousarray(
            Wv.T.reshape(DIM // 128, 128, 128)
        ).astype(ml_dtypes.bfloat16)
        bias_np = np.ascontiguousarray(
            np.stack([b[r0 : r0 + 128], b[DIM + r0 : DIM + r0 + 128]], axis=1)
        )
        in_maps.append(
            {"xt": xt_np, "wqk": wqk_np, "wv": wv_np, "bias": bias_np}
        )
    meta = {"pad_idx": pad_idx, "nnz": nnz, "bv": b[2 * DIM :]}
    return nc, in_maps, meta


def _postprocess(raws, meta):
    """raws: list of per-core [130, nnz_pad] f32 -> full (nnz, 1024) f32."""
    pad_idx = meta["pad_idx"]
    nnz = meta["nnz"]
    bv = meta["bv"]
    out = np.empty((nnz, DIM), dtype=np.float32)
    for c in range(N_CORES):
        raw = np.asarray(raws[c], dtype=np.float32)
        for h in range(HEADS_PER_CORE):
            num = raw[65 * h : 65 * h + 64][:, pad_idx]  # (64, nnz)
            den = raw[65 * h + 64][pad_idx]  # (nnz,)
            col = c * 128 + h * 64
            out[:, col : col + 64] = num.T / den[:, None] + bv[col : col + 64]
    return out


def kernel(hidden_states, Wqkv_weight, Wqkv_bias, cu_seqlens, max_seqlen=None):
    from concourse.bass_utils import run_bass_kernel_spmd

    nc, in_maps, meta = _prepare(
        hidden_states, Wqkv_weight, Wqkv_bias, cu_seqlens
    )
    res = run_bass_kernel_spmd(nc, in_maps, list(range(N_CORES)))
    return _postprocess(
        [res.results[c]["out"] for c in range(N_CORES)], meta
    )


# revision 36
# speedup vs baseline: 1.0449x; 1.0431x over previous
"""Trainium2 Bass kernel for varlen (ragged) BERT self-attention.

Strategy: tensor-parallel over heads. 16 heads across 8 NeuronCores ->
2 heads per core. Every core runs an IDENTICAL program (SPMD) on:
  - xt:   full hidden_states, host-padded so each sequence starts at a
          128-aligned token offset, transposed + bf16, laid out
          [128, n_ti, 8, 512] so each 512-token chunk is one contiguous
          8KB-per-partition DMA read.
  - wqk:  this core's q/k weight slices as matmul-lhsT blocks
          [8, 128, 256] bf16 (cols 0:128 = q, 128:256 = k).
  - wv:   this core's v weight slice as matmul-rhs blocks [8, 128, 128].
  - bias: q/k bias [128, 2] f32 (per-partition adds on eviction).
Output per core: raw [130, nnz_pad] f32: rows 0:64   = sum_k p*v (head0)
                                         row  64    = sum_k p   (head0)
                                         rows 65:129/129 same for head1.
Host divides by the denominator row, adds the v bias (algebraically
exact: softmax(s) @ (v+bv) = softmax(s)@v + bv), and gathers the
padded layout back to the ragged one.

On-chip per core:
  1. QKV projection, K=1024 in 8 chunks of 128:
     - q,k computed TRANSPOSED (stationary = W slices): qT/kT
       [128(=2h x 64d), nnz_pad] bf16, bias added during PSUM->SBUF
       eviction (DVE tensor_scalar add, per-partition).
     - v computed NATURAL (stationary = xt chunks, moving = Wv):
       v_nat [128(tok), chunk, 2, 65] bf16 with a preset ones column;
       no transposes needed anywhere.
  2. Attention per sequence (no packing; every sequence 128-aligned):
     scores[k,q] = kT.T @ qT per head with K=64 -> the two heads'
     matmuls auto-place on PE row tiles (0,0)/(64,0) and run
     CONCURRENTLY. exp on ACT over both heads in one instruction
     (scale=1/8 folded in). outT[65, q] accumulated as
     (v|ones).T @ exp with K<=128; the ones column gives the softmax
     denominator for free. PSUM -> SBUF copy (DVE) -> DMA out raw.

Emission interleaves per-jc attention into the QKV chunk stream
(chunks processed back-to-front, sequences emitted largest-first) so
the PE instruction stream stays dense end-to-end (HAM clock gate).
"""

import functools
import sys

import numpy as np

for _p in ("/opt/trn_rl_repo",):
    if _p not in sys.path:
        sys.path.append(_p)

import ml_dtypes  # noqa: E402

N_HEADS = 16
HEAD_DIM = 64
DIM = 1024
N_CORES = 8
HEADS_PER_CORE = N_HEADS // N_CORES  # 2


def _padded_units(lengths):
    """One unit per sequence at a 128-aligned padded offset."""
    units = []
    off = 0
    for L in lengths:
        if L == 0:
            continue
        units.append((off, L))
        off += ((L + 127) // 128) * 128
    nnz_pad = ((off + 511) // 512) * 512
    return tuple(units), nnz_pad


@functools.lru_cache(maxsize=4)
def _build(nnz_pad, units):
    """Build + compile the SPMD Bass program for the given ragged lengths."""
    import concourse.mybir as mybir
    import concourse.tile as tile
    from concourse import bacc

    f32 = mybir.dt.float32
    bf16 = mybir.dt.bfloat16
    Exp = mybir.ActivationFunctionType.Exp

    KC = DIM // 128  # 8 contraction chunks
    D = HEAD_DIM
    n_ti = nnz_pad // 512
    n_ch = nnz_pad // 128

    nc = bacc.Bacc("TRN2", target_bir_lowering=False, debug=False)
    xt = nc.declare_dram_parameter("xt", [128, n_ti, KC, 512], bf16, isOutput=False)
    wqk = nc.declare_dram_parameter("wqk", [KC, 128, 256], bf16, isOutput=False)
    wv = nc.declare_dram_parameter("wv", [KC, 128, 128], bf16, isOutput=False)
    bias = nc.declare_dram_parameter("bias", [128, 2], f32, isOutput=False)
    out = nc.declare_dram_parameter("out", [130, nnz_pad], f32, isOutput=True)

    with tile.TileContext(nc) as tc:
        with (
            tc.tile_pool(name="res", bufs=1) as res,
            tc.tile_pool(name="xp", bufs=4) as xp,
            tc.tile_pool(name="esp", bufs=26) as esp,
            tc.tile_pool(name="otp", bufs=4) as otp,
            tc.tile_pool(name="ps", bufs=1, space="PSUM") as ps,
        ):
            # --- resident tensors; startup DMAs split per-chunk and spread
            # across engine queues so the first matmul starts ASAP ---
            wqk_sb = res.tile([128, KC, 256], bf16)
            wv_sb = res.tile([128, KC, 128], bf16)
            bias_sb = res.tile([128, 2], f32)
            qT = res.tile([128, nnz_pad], bf16)
            kT = res.tile([128, nnz_pad], bf16)
            # v in natural layout: [tok(128), chunk, head, 64 v + 1 ones]
            v_nat = res.tile([128, n_ch, 2, D + 1], bf16)
            # HAM warm-up: dummy matmuls on a memset tile during the initial
            # DMA wait release the PE clock gate before real work arrives
            wrm = res.tile([128, 512], bf16)
            nc.vector.memset(wrm[:, :], 0.0)
            nc.gpsimd.memset(v_nat[:, :, :, D : D + 1], 1.0)
            xt_tiles = {}

            def ensure_xt(ti, split=False, eng=None):
                if ti >= n_ti or ti < 0 or ti in xt_tiles:
                    return
                t = xp.tile([128, KC, 512], bf16, tag="xt", name="xt_t")
                if split:
                    for kc in range(KC):
                        e = (nc.gpsimd if kc < 4 else nc.scalar) if eng is None else eng
                        e.dma_start(t[:, kc, :], xt[:, ti, kc, :])
                else:
                    (eng or nc.sync).dma_start(t[:], xt[:, ti, :, :])
                xt_tiles[ti] = t

            for kc in range(KC):
                nc.sync.dma_start(
                    wqk_sb[:, kc, :], wqk[kc, :, :]
                )
            ensure_xt(n_ti - 1, split=True)
            nc.scalar.dma_start(bias_sb[:], bias[:, :])
            nc.scalar.dma_start(
                wv_sb[:], wv[:, :, :].rearrange("a p m -> p a m")
            )
            for _ in range(7):
                dm = ps.tile([128, 512], f32, tag="mm", bufs=2, name="dm")
                nc.tensor.matmul(
                    dm[:, :], wrm[:, 0:128], wrm[:, :], start=True, stop=True
                )
            ensure_xt(n_ti - 2)

            # --- QKV feeder: yields 3 groups per token chunk (q, k+v0, v1)
            # so attention emission can interleave dense PE work ---
            state = {"ti_next": n_ti}

            def _qkv_groups():
                for ti in range(n_ti - 1, -1, -1):
                    t0 = ti * 512
                    ensure_xt(ti)
                    ensure_xt(ti - 1)
                    ensure_xt(ti - 2)
                    xt_tile = xt_tiles.pop(ti)
                    # q group: stationary W, moving xt
                    mmq = ps.tile([128, 512], f32, tag="mm", bufs=2, name="mmq")
                    for kc in range(KC):
                        nc.tensor.matmul(
                            mmq[:, :],
                            wqk_sb[:, kc, 0:128],
                            xt_tile[:, kc, :],
                            start=(kc == 0),
                            stop=(kc == KC - 1),
                        )
                    nc.vector.tensor_scalar_add(
                        qT[:, t0 : t0 + 512], mmq[:, :], bias_sb[:, 0:1]
                    )
                    yield
                    # k group interleaved with first half of v (v: stationary
                    # xt chunk, moving Wv -> LDW-heavy; hide under k streams)
                    mmk = ps.tile([128, 512], f32, tag="mm", bufs=2, name="mmk")
                    mmv = ps.tile([128, 4, 2, D], f32, tag="mm", bufs=2, name="mmv")
                    vs = [(c, kc) for c in range(4) for kc in range(KC)]
                    vi = 0

                    def _vmm(c, kc):
                        nc.tensor.matmul(
                            mmv[:, c, :, :],
                            xt_tile[:, kc, c * 128 : (c + 1) * 128],
                            wv_sb[:, kc, :],
                            start=(kc == 0),
                            stop=(kc == KC - 1),
                        )

                    for kc in range(KC):
                        nc.tensor.matmul(
                            mmk[:, :],
                            wqk_sb[:, kc, 128:256],
                            xt_tile[:, kc, :],
                            start=(kc == 0),
                            stop=(kc == KC - 1),
                        )
                        for _ in range(2):
                            _vmm(*vs[vi])
                            vi += 1
                    nc.vector.tensor_scalar_add(
                        kT[:, t0 : t0 + 512], mmk[:, :], bias_sb[:, 1:2]
                    )
                    yield
                    # rest of v
                    while vi < len(vs):
                        _vmm(*vs[vi])
                        vi += 1
                    nc.vector.tensor_copy(
                        v_nat[:, ti * 4 : ti * 4 + 4, :, 0:D], mmv[:, :, :, :]
                    )
                    state["ti_next"] = ti
                    yield

            feeder = _qkv_groups()

            # pacing: spread remaining feeder groups over remaining
            # attention jc-iterations (recomputed each step)
            n_groups = 3 * n_ti
            n_iters = sum(
                ((L + 127) // 128) * ((L + 511) // 512) for _, L in units
            )
            pace = {"acc": 0.0, "groups": n_groups, "iters": n_iters}

            def feed(n):
                for _ in range(n):
                    if next(feeder, "done") == "done":
                        break
                    pace["groups"] -= 1

            def feed_cb():
                if pace["iters"] > 0:
                    pace["acc"] += pace["groups"] / pace["iters"]
                pace["iters"] -= 1
                k = min(int(pace["acc"]), pace["groups"])
                if k > 0:
                    pace["acc"] -= k
                    feed(k)
                else:
                    # no feeder work this slot: cheap PE keepalive so the
                    # HAM clock gate stays released through ACT-bound slots.
                    # K=64, M=128 keeps it in the attention tiling mode
                    # (no mode-switch drain on either side).
                    dm = ps.tile([128, 64], f32, tag="mm", bufs=2, name="dm")
                    nc.tensor.matmul(
                        dm[:, :], wrm[0:64, 0:128], wrm[0:64, 0:64],
                        start=True, stop=True,
                    )

            # --- deferred out-matmul machinery -------------------------------
            # Out matmuls run as 64-row-mode split-K pairs (tokens 0:64 on PE
            # row tile (0,0), tokens 64:128 on (64,0)) so the whole attention
            # stream stays in one PE tiling mode (mode switches cost ~300ns).
            # Each (qc, head) package accumulates into ov[:, 0/1, :] (a/b
            # banks); the a+b merge happens in the DVE eviction for free.
            # Packages are queued and drained during the FOLLOWING qc's score
            # slots, overlapping the ACT-paced exp stream.
            outq = []  # pending out-pairs: (emit_fn,)

            def drain_out(k):
                while k > 0 and outq:
                    outq.pop(0)()
                    k -= 1

            def push_pkg(entries, dmas):
                """Queue out-matmuls + eviction for one package; both heads.

                entries: [(gb, jc_global_chunk, es, nj, off, nq_u)] — each
                contributes its unit's k-chunk into packed ov columns
                [off, off+nq_u). dmas: [(q0_hbm, off, nq_u)].
                width: total packed columns (<= 512).
                """
                width = max(e[4] + e[5] for e in entries)
                bids = [i for i, e in enumerate(entries) if e[3] > 64]
                # column ranges whose b-half bank actually gets written --
                # only those may be merged in (elsewhere the bank is stale)
                bcols = sorted({(e[4], e[5]) for e in entries if e[3] > 64})
                merged = []
                for off_u, n_u in bcols:
                    if merged and merged[-1][0] + merged[-1][1] == off_u:
                        merged[-1] = (merged[-1][0], merged[-1][1] + n_u)
                    else:
                        merged.append((off_u, n_u))
                for h in range(2):
                    ov = ps.tile(
                        [D + 1, 2, 512], f32, tag="ov", bufs=1, name="ov"
                    )

                    def mk(i, e, h=h, ov=ov):
                        gc, es, nj, off, nq_u = e[1], e[2], e[3], e[4], e[5]

                        def go():
                            na = min(nj, 64)
                            nc.tensor.matmul(
                                ov[:, 0, off : off + nq_u],
                                v_nat[0:na, gc, h, :],
                                es[0:na, h, :nq_u],
                                start=(i == 0),
                                stop=(i == len(entries) - 1),
                            )
                            if nj > 64:
                                nc.tensor.matmul(
                                    ov[:, 1, off : off + nq_u],
                                    v_nat[64:nj, gc, h, :],
                                    es[64:nj, h, :nq_u],
                                    start=(i == bids[0]),
                                    stop=(i == bids[-1]),
                                )

                        return go

                    for i, e in enumerate(entries):
                        outq.append(mk(i, e))

                    def evict(h=h, ov=ov, oc=dmas):
                        ot = otp.tile(
                            [D + 1, 2, 512], f32, tag="ot", name=f"ot{h}"
                        )
                        nc.vector.tensor_copy(
                            ot[:, h, :width], ov[:, 0, :width]
                        )
                        for off_u, n_u in merged:
                            nc.vector.tensor_add(
                                ot[:, h, off_u : off_u + n_u],
                                ot[:, h, off_u : off_u + n_u],
                                ov[:, 1, off_u : off_u + n_u],
                            )
                        for q0, off, nq_u in oc:
                            nc.sync.dma_start(
                                out[
                                    65 * h : 65 * h + 65, q0 : q0 + nq_u
                                ],
                                ot[:, h, off : off + nq_u],
                            )

                    outq.append(evict)

            def emit_scores(O, L, q0, nq, jcs):
                """Score+exp slots for one unit's (qc); returns es entries."""
                gb = O // 128
                nk = (L + 127) // 128
                out_entries = []
                for jc in jcs:
                    feed_cb()
                    nj = min(128, L - jc * 128)
                    k0 = O + jc * 128
                    sps = ps.tile(
                        [128, 2, 512], f32, tag="sc", bufs=2, name="sps"
                    )
                    # two heads on PE row tiles (0,0) / (64,0): concurrent
                    for h in range(2):
                        p0 = D * h
                        nc.tensor.matmul(
                            sps[:nj, h, :nq],
                            kT[p0 : p0 + D, k0 : k0 + nj],
                            qT[p0 : p0 + D, q0 : q0 + nq],
                            start=True,
                            stop=True,
                        )
                    es = esp.tile([128, 2, 512], bf16, tag="es", name="es")
                    nc.scalar.activation(
                        es[:nj, :, :nq], sps[:nj, :, :nq], Exp, scale=0.125
                    )
                    out_entries.append((gb, gb + jc, es, nj))
                    # keep the out-queue drained by the time slots run out
                    k = 3
                    if 0 < pace["iters"] < 40:
                        k = min(6, max(3, -(-len(outq) // pace["iters"])))
                    drain_out(k)
                return out_entries

            def emit_attention(O, L):
                nk = (L + 127) // 128
                for qc in range((L + 511) // 512):
                    q0 = O + qc * 512
                    nq = min(512, L - qc * 512)
                    ents = emit_scores(O, L, q0, nq, range(nk))
                    push_pkg(
                        [(gb, gc, es, nj, 0, nq) for gb, gc, es, nj in ents],
                        [(q0, 0, nq)],
                    )

            def emit_attention_group(grp):
                """Several small units (each L<=512, total<=512) share one
                package: their out columns pack side by side in one ov."""
                entries, dmas = [], []
                off = 0
                for O, L in grp:
                    nk = (L + 127) // 128
                    ents = emit_scores(O, L, O, L, range(nk))
                    entries.extend(
                        (gb, gc, es, nj, off, L) for gb, gc, es, nj in ents
                    )
                    dmas.append((O, off, L))
                    off += L
                push_pkg(entries, dmas)

            # --- interleaved emission: chunks back-to-front via the feeder;
            # a unit is ready once all chunks covering [O, O+L) are emitted.
            pending = sorted(units, key=lambda u: u[0], reverse=True)
            i = 0
            while i < len(pending):
                O, L = pending[i]
                if L >= 512:
                    while state["ti_next"] * 512 > O:
                        feed(1)
                    emit_attention(O, L)
                    i += 1
                else:
                    grp, tot = [], 0
                    while (
                        i < len(pending)
                        and pending[i][1] < 512
                        and tot + pending[i][1] <= 512
                    ):
                        grp.append(pending[i])
                        tot += pending[i][1]
                        i += 1
                    while state["ti_next"] * 512 > grp[-1][0]:
                        feed(1)
                    emit_attention_group(grp)
            while outq:
                outq.pop(0)()
                feed_cb()  # keepalive through the final drain
            feed(n_groups)  # drain any leftovers

    nc.compile()
    return nc


def _prepare(hidden_states, Wqkv_weight, Wqkv_bias, cu_seqlens):
    """Host-side sharding prep. Returns (nc, in_maps, meta)."""
    hs = np.asarray(hidden_states, dtype=np.float32)
    W = np.asarray(Wqkv_weight, dtype=np.float32)
    b = np.asarray(Wqkv_bias, dtype=np.float32).reshape(-1)
    cs = np.asarray(cu_seqlens).astype(np.int64).reshape(-1)
    nnz, dim = hs.shape
    assert dim == DIM and W.shape == (3 * DIM, DIM)
    lengths = tuple(int(cs[i + 1] - cs[i]) for i in range(len(cs) - 1))
    assert sum(lengths) == nnz, (lengths, nnz)

    units, nnz_pad = _padded_units(lengths)
    nc = _build(nnz_pad, units)

    # padded token index map: real token t -> padded column pad_idx[t]
    pad_idx = np.empty(nnz, dtype=np.int64)
    t = 0
    for (O, L) in units:
        pad_idx[t : t + L] = np.arange(O, O + L)
        t += L

    xt_pad = np.zeros((DIM, nnz_pad), dtype=np.float32)
    xt_pad[:, pad_idx] = hs.T
    n_ti = nnz_pad // 512
    xt_np = np.ascontiguousarray(
        xt_pad.reshape(DIM // 128, 128, n_ti, 512).transpose(1, 2, 0, 3)
    ).astype(ml_dtypes.bfloat16)

    in_maps = []
    for c in range(N_CORES):
        r0 = c * HEADS_PER_CORE * HEAD_DIM  # 128c
        Wq = W[r0 : r0 + 128, :]
        Wk = W[DIM + r0 : DIM + r0 + 128, :]
        Wv = W[2 * DIM + r0 : 2 * DIM + r0 + 128, :]
        wqk_np = np.ascontiguousarray(
            np.concatenate([Wq.T, Wk.T], axis=1).reshape(DIM // 128, 128, 256)
        ).astype(ml_dtypes.bfloat16)
        wv_np = np.ascontiguousarray(
            Wv.T.reshape(DIM // 128, 128, 128)
        ).astype(ml_dtypes.bfloat16)
        bias_np = np.ascontiguousarray(
            np.stack([b[r0 : r0 + 128], b[DIM + r0 : DIM + r0 + 128]], axis=1)
        )
        in_maps.append(
            {"xt": xt_np, "wqk": wqk_np, "wv": wv_np, "bias": bias_np}
        )
    meta = {"pad_idx": pad_idx, "nnz": nnz, "bv": b[2 * DIM :]}
    return nc, in_maps, meta


def _postprocess(raws, meta):
    """raws: list of per-core [130, nnz_pad] f32 -> full (nnz, 1024) f32."""
    pad_idx = meta["pad_idx"]
    nnz = meta["nnz"]
    bv = meta["bv"]
    out = np.empty((nnz, DIM), dtype=np.float32)
    for c in range(N_CORES):
        raw = np.asarray(raws[c], dtype=np.float32)
        for h in range(HEADS_PER_CORE):
            num = raw[65 * h : 65 * h + 64][:, pad_idx]  # (64, nnz)
            den = raw[65 * h + 64][pad_idx]  # (nnz,)
            col = c * 128 + h * 64
            out[:, col : col + 64] = num.T / den[:, None] + bv[col : col + 64]
    return out


def kernel(hidden_states, Wqkv_weight, Wqkv_bias, cu_seqlens, max_seqlen=None):
    from concourse.bass_utils import run_bass_kernel_spmd

    nc, in_maps, meta = _prepare(
        hidden_states, Wqkv_weight, Wqkv_bias, cu_seqlens
    )
    res = run_bass_kernel_spmd(nc, in_maps, list(range(N_CORES)))
    return _postprocess(
        [res.results[c]["out"] for c in range(N_CORES)], meta
    )


# revision 43
# speedup vs baseline: 1.0491x; 1.0040x over previous
"""Trainium2 Bass kernel for varlen (ragged) BERT self-attention.

Strategy: tensor-parallel over heads. 16 heads across 8 NeuronCores ->
2 heads per core. Every core runs an IDENTICAL program (SPMD) on:
  - xt:   full hidden_states, host-padded so each sequence starts at a
          128-aligned token offset, transposed + bf16, laid out
          [128, n_ti, 8, 512] so each 512-token chunk is one contiguous
          8KB-per-partition DMA read.
  - wqk:  this core's q/k weight slices as matmul-lhsT blocks
          [8, 128, 256] bf16 (cols 0:128 = q, 128:256 = k).
  - wv:   this core's v weight slice as matmul-rhs blocks [8, 128, 128].
  - bias: q/k bias [128, 2] f32 (per-partition adds on eviction).
Output per core: raw [130, nnz_pad] f32: rows 0:64   = sum_k p*v (head0)
                                         row  64    = sum_k p   (head0)
                                         rows 65:129/129 same for head1.
Host divides by the denominator row, adds the v bias (algebraically
exact: softmax(s) @ (v+bv) = softmax(s)@v + bv), and gathers the
padded layout back to the ragged one.

On-chip per core:
  1. QKV projection, K=1024 in 8 chunks of 128:
     - q,k computed TRANSPOSED (stationary = W slices): qT/kT
       [128(=2h x 64d), nnz_pad] bf16, bias added during PSUM->SBUF
       eviction (DVE tensor_scalar add, per-partition).
     - v computed NATURAL (stationary = xt chunks, moving = Wv):
       v_nat [128(tok), chunk, 2, 65] bf16 with a preset ones column;
       no transposes needed anywhere.
  2. Attention per sequence (no packing; every sequence 128-aligned):
     scores[k,q] = kT.T @ qT per head with K=64 -> the two heads'
     matmuls auto-place on PE row tiles (0,0)/(64,0) and run
     CONCURRENTLY. exp on ACT over both heads in one instruction
     (scale=1/8 folded in). outT[65, q] accumulated as
     (v|ones).T @ exp with K<=128; the ones column gives the softmax
     denominator for free. PSUM -> SBUF copy (DVE) -> DMA out raw.

Emission interleaves per-jc attention into the QKV chunk stream
(chunks processed back-to-front, sequences emitted largest-first) so
the PE instruction stream stays dense end-to-end (HAM clock gate).
"""

import functools
import sys

import numpy as np

for _p in ("/opt/trn_rl_repo",):
    if _p not in sys.path:
        sys.path.append(_p)

import ml_dtypes  # noqa: E402

N_HEADS = 16
HEAD_DIM = 64
DIM = 1024
N_CORES = 8
HEADS_PER_CORE = N_HEADS // N_CORES  # 2


def _padded_units(lengths):
    """One unit per sequence at a 128-aligned padded offset."""
    units = []
    off = 0
    for L in lengths:
        if L == 0:
            continue
        units.append((off, L))
        off += ((L + 127) // 128) * 128
    nnz_pad = ((off + 511) // 512) * 512
    return tuple(units), nnz_pad


@functools.lru_cache(maxsize=4)
def _build(nnz_pad, units):
    """Build + compile the SPMD Bass program for the given ragged lengths."""
    import concourse.mybir as mybir
    import concourse.tile as tile
    from concourse import bacc

    f32 = mybir.dt.float32
    bf16 = mybir.dt.bfloat16
    Exp = mybir.ActivationFunctionType.Exp

    KC = DIM // 128  # 8 contraction chunks
    D = HEAD_DIM
    n_ti = nnz_pad // 512
    n_ch = nnz_pad // 128

    nc = bacc.Bacc("TRN2", target_bir_lowering=False, debug=False)
    xt = nc.declare_dram_parameter("xt", [128, n_ti, KC, 512], bf16, isOutput=False)
    wqk = nc.declare_dram_parameter("wqk", [KC, 128, 256], bf16, isOutput=False)
    wv = nc.declare_dram_parameter("wv", [KC, 128, 128], bf16, isOutput=False)
    bias = nc.declare_dram_parameter("bias", [128, 2], f32, isOutput=False)
    out = nc.declare_dram_parameter("out", [130, nnz_pad], f32, isOutput=True)

    with tile.TileContext(nc) as tc:
        with (
            tc.tile_pool(name="res", bufs=1) as res,
            tc.tile_pool(name="xp", bufs=4) as xp,
            tc.tile_pool(name="esp", bufs=26) as esp,
            tc.tile_pool(name="otp", bufs=4) as otp,
            tc.tile_pool(name="ps", bufs=1, space="PSUM") as ps,
        ):
            # --- resident tensors; startup DMAs split per-chunk and spread
            # across engine queues so the first matmul starts ASAP ---
            wqk_sb = res.tile([128, KC, 256], bf16)
            wv_sb = res.tile([128, KC, 128], bf16)
            bias_sb = res.tile([128, 2], f32)
            qT = res.tile([128, nnz_pad], bf16)
            kT = res.tile([128, nnz_pad], bf16)
            # v in natural layout: [tok(128), chunk, head, 64 v + 1 ones]
            v_nat = res.tile([128, n_ch, 2, D + 1], bf16)
            # HAM warm-up: dummy matmuls on a memset tile during the initial
            # DMA wait release the PE clock gate before real work arrives
            wrm = res.tile([128, 512], bf16)
            nc.vector.memset(wrm[:, :], 0.0)
            nc.gpsimd.memset(v_nat[:, :, :, D : D + 1], 1.0)
            xt_tiles = {}

            def ensure_xt(ti, split=False, eng=None):
                if ti >= n_ti or ti < 0 or ti in xt_tiles:
                    return
                t = xp.tile([128, KC, 512], bf16, tag="xt", name="xt_t")
                if split:
                    for kc in range(KC):
                        e = (nc.gpsimd if kc < 4 else nc.scalar) if eng is None else eng
                        e.dma_start(t[:, kc, :], xt[:, ti, kc, :])
                else:
                    (eng or nc.sync).dma_start(t[:], xt[:, ti, :, :])
                xt_tiles[ti] = t

            for kc in range(KC):
                nc.sync.dma_start(
                    wqk_sb[:, kc, :], wqk[kc, :, :]
                )
            ensure_xt(n_ti - 1, split=True)
            nc.scalar.dma_start(bias_sb[:], bias[:, :])
            nc.scalar.dma_start(
                wv_sb[:], wv[:, :, :].rearrange("a p m -> p a m")
            )
            for _ in range(9):
                dm = ps.tile([128, 512], f32, tag="mm", bufs=2, name="dm")
                nc.tensor.matmul(
                    dm[:, :], wrm[:, 0:128], wrm[:, :], start=True, stop=True
                )
            ensure_xt(n_ti - 2, split=True, eng=nc.sync)

            # --- QKV feeder: yields 3 groups per token chunk (q, k+v0, v1)
            # so attention emission can interleave dense PE work ---
            state = {"ti_next": n_ti}

            def _qkv_groups():
                for ti in range(n_ti - 1, -1, -1):
                    t0 = ti * 512
                    ensure_xt(ti)
                    ensure_xt(ti - 1)
                    ensure_xt(ti - 2)
                    xt_tile = xt_tiles.pop(ti)
                    # q group: stationary W, moving xt
                    mmq = ps.tile([128, 512], f32, tag="mm", bufs=2, name="mmq")
                    for kc in range(KC):
                        nc.tensor.matmul(
                            mmq[:, :],
                            wqk_sb[:, kc, 0:128],
                            xt_tile[:, kc, :],
                            start=(kc == 0),
                            stop=(kc == KC - 1),
                        )
                    nc.vector.tensor_scalar_add(
                        qT[:, t0 : t0 + 512], mmq[:, :], bias_sb[:, 0:1]
                    )
                    yield
                    # k group interleaved with first half of v (v: stationary
                    # xt chunk, moving Wv -> LDW-heavy; hide under k streams)
                    mmk = ps.tile([128, 512], f32, tag="mm", bufs=2, name="mmk")
                    mmv = ps.tile([128, 4, 2, D], f32, tag="mm", bufs=2, name="mmv")
                    vs = [(c, kc) for c in range(4) for kc in range(KC)]
                    vi = 0

                    def _vmm(c, kc):
                        nc.tensor.matmul(
                            mmv[:, c, :, :],
                            xt_tile[:, kc, c * 128 : (c + 1) * 128],
                            wv_sb[:, kc, :],
                            start=(kc == 0),
                            stop=(kc == KC - 1),
                        )

                    for kc in range(KC):
                        nc.tensor.matmul(
                            mmk[:, :],
                            wqk_sb[:, kc, 128:256],
                            xt_tile[:, kc, :],
                            start=(kc == 0),
                            stop=(kc == KC - 1),
                        )
                        for _ in range(2):
                            _vmm(*vs[vi])
                            vi += 1
                    nc.vector.tensor_scalar_add(
                        kT[:, t0 : t0 + 512], mmk[:, :], bias_sb[:, 1:2]
                    )
                    yield
                    # rest of v
                    while vi < len(vs):
                        _vmm(*vs[vi])
                        vi += 1
                    nc.vector.tensor_copy(
                        v_nat[:, ti * 4 : ti * 4 + 4, :, 0:D], mmv[:, :, :, :]
                    )
                    state["ti_next"] = ti
                    yield

            feeder = _qkv_groups()

            # pacing: spread remaining feeder groups over remaining
            # attention jc-iterations (recomputed each step)
            n_groups = 3 * n_ti
            n_iters = sum(
                ((L + 127) // 128) * ((L + 511) // 512) for _, L in units
            )
            pace = {"acc": 0.0, "groups": n_groups, "iters": n_iters}

            def feed(n):
                for _ in range(n):
                    if next(feeder, "done") == "done":
                        break
                    pace["groups"] -= 1

            def feed_cb():
                if pace["iters"] > 0:
                    pace["acc"] += pace["groups"] / pace["iters"]
                pace["iters"] -= 1
                k = min(int(pace["acc"]), pace["groups"])
                if k > 0:
                    pace["acc"] -= k
                    feed(k)
                elif outq:
                    # no feeder work this slot: drain a real out-pair --
                    # useful PE work doubles as the HAM keepalive
                    drain_out(1)
                else:
                    # queue empty too: cheap PE keepalive so the HAM clock
                    # gate stays released through ACT-bound slots. K=64,
                    # M=128 keeps it in the attention tiling mode.
                    dm = ps.tile([128, 64], f32, tag="mm", bufs=2, name="dm")
                    nc.tensor.matmul(
                        dm[:, :], wrm[0:64, 0:128], wrm[0:64, 0:64],
                        start=True, stop=True,
                    )

            # --- deferred out-matmul machinery -------------------------------
            # Out matmuls run as 64-row-mode split-K pairs (tokens 0:64 on PE
            # row tile (0,0), tokens 64:128 on (64,0)) so the whole attention
            # stream stays in one PE tiling mode (mode switches cost ~300ns).
            # Each (qc, head) package accumulates into ov[:, 0/1, :] (a/b
            # banks); the a+b merge happens in the DVE eviction for free.
            # Packages are queued and drained during the FOLLOWING qc's score
            # slots, overlapping the ACT-paced exp stream.
            outq = []  # pending out-pairs: (emit_fn,)

            def drain_out(k):
                while k > 0 and outq:
                    outq.pop(0)()
                    k -= 1

            def push_pkg(entries, dmas):
                """Queue out-matmuls + eviction for one package; both heads.

                entries: [(gb, jc_global_chunk, es, nj, off, nq_u)] — each
                contributes its unit's k-chunk into packed ov columns
                [off, off+nq_u). dmas: [(q0_hbm, off, nq_u)].
                width: total packed columns (<= 512).
                """
                width = max(e[4] + e[5] for e in entries)
                bids = [i for i, e in enumerate(entries) if e[3] > 64]
                # column ranges whose b-half bank actually gets written --
                # only those may be merged in (elsewhere the bank is stale)
                bcols = sorted({(e[4], e[5]) for e in entries if e[3] > 64})
                merged = []
                for off_u, n_u in bcols:
                    if merged and merged[-1][0] + merged[-1][1] == off_u:
                        merged[-1] = (merged[-1][0], merged[-1][1] + n_u)
                    else:
                        merged.append((off_u, n_u))
                for h in range(2):
                    ov = ps.tile(
                        [D + 1, 2, 512], f32, tag="ov", bufs=1, name="ov"
                    )

                    def mk(i, e, h=h, ov=ov):
                        gc, es, nj, off, nq_u = e[1], e[2], e[3], e[4], e[5]

                        def go():
                            na = min(nj, 64)
                            nc.tensor.matmul(
                                ov[:, 0, off : off + nq_u],
                                v_nat[0:na, gc, h, :],
                                es[0:na, h, :nq_u],
                                start=(i == 0),
                                stop=(i == len(entries) - 1),
                            )
                            if nj > 64:
                                nc.tensor.matmul(
                                    ov[:, 1, off : off + nq_u],
                                    v_nat[64:nj, gc, h, :],
                                    es[64:nj, h, :nq_u],
                                    start=(i == bids[0]),
                                    stop=(i == bids[-1]),
                                )

                        return go

                    for i, e in enumerate(entries):
                        outq.append(mk(i, e))

                    def evict(h=h, ov=ov, oc=dmas):
                        dmaq = (nc.sync, nc.scalar, nc.gpsimd)
                        ot = otp.tile(
                            [D + 1, 2, 512], f32, tag="ot", name=f"ot{h}"
                        )
                        nc.vector.tensor_copy(
                            ot[:, h, :width], ov[:, 0, :width]
                        )
                        for off_u, n_u in merged:
                            nc.vector.tensor_add(
                                ot[:, h, off_u : off_u + n_u],
                                ot[:, h, off_u : off_u + n_u],
                                ov[:, 1, off_u : off_u + n_u],
                            )
                        for di, (q0, off, nq_u) in enumerate(oc):
                            dmaq[(di + h) % 3].dma_start(
                                out[
                                    65 * h : 65 * h + 65, q0 : q0 + nq_u
                                ],
                                ot[:, h, off : off + nq_u],
                            )

                    outq.append(evict)

            def emit_scores(O, L, q0, nq, jcs):
                """Score+exp slots for one unit's (qc); returns es entries."""
                gb = O // 128
                nk = (L + 127) // 128
                out_entries = []
                for jc in jcs:
                    feed_cb()
                    nj = min(128, L - jc * 128)
                    k0 = O + jc * 128
                    sps = ps.tile(
                        [128, 2, 512], f32, tag="sc", bufs=2, name="sps"
                    )
                    # two heads on PE row tiles (0,0) / (64,0): concurrent
                    for h in range(2):
                        p0 = D * h
                        nc.tensor.matmul(
                            sps[:nj, h, :nq],
                            kT[p0 : p0 + D, k0 : k0 + nj],
                            qT[p0 : p0 + D, q0 : q0 + nq],
                            start=True,
                            stop=True,
                        )
                    es = esp.tile([128, 2, 512], bf16, tag="es", name="es")
                    nc.scalar.activation(
                        es[:nj, :, :nq], sps[:nj, :, :nq], Exp, scale=0.125
                    )
                    out_entries.append((gb, gb + jc, es, nj))
                    # keep the out-queue drained by the time slots run out
                    k = 3
                    if 0 < pace["iters"] < 40:
                        k = min(6, max(3, -(-len(outq) // pace["iters"])))
                    drain_out(k)
                return out_entries

            def emit_attention(O, L):
                nk = (L + 127) // 128
                for qc in range((L + 511) // 512):
                    q0 = O + qc * 512
                    nq = min(512, L - qc * 512)
                    ents = emit_scores(O, L, q0, nq, range(nk))
                    push_pkg(
                        [(gb, gc, es, nj, 0, nq) for gb, gc, es, nj in ents],
                        [(q0, 0, nq)],
                    )

            def emit_attention_group(grp):
                """Several small units (each L<=512, total<=512) share one
                package: their out columns pack side by side in one ov."""
                entries, dmas = [], []
                off = 0
                for O, L in grp:
                    nk = (L + 127) // 128
                    ents = emit_scores(O, L, O, L, range(nk))
                    entries.extend(
                        (gb, gc, es, nj, off, L) for gb, gc, es, nj in ents
                    )
                    dmas.append((O, off, L))
                    off += L
                push_pkg(entries, dmas)

            # --- interleaved emission: chunks back-to-front via the feeder;
            # a unit is ready once all chunks covering [O, O+L) are emitted.
            pending = sorted(units, key=lambda u: u[0], reverse=True)
            sched = []
            i = 0
            while i < len(pending):
                O, L = pending[i]
                if L >= 512:
                    sched.append([(O, L)])
                    i += 1
                else:
                    grp, tot = [], 0
                    while (
                        i < len(pending)
                        and pending[i][1] < 512
                        and tot + pending[i][1] <= 512
                    ):
                        grp.append(pending[i])
                        tot += pending[i][1]
                        i += 1
                    sched.append(grp)
            if len(sched) >= 2 and len(sched[-1]) > 1 and len(sched[-2]) > 1:
                # the final two small-seq groups become ready together; put
                # the smaller one first so the larger group's slots absorb
                # its out-package drain instead of a post-slot serial tail
                sched[-1], sched[-2] = sched[-2], sched[-1]
            for grp in sched:
                while state["ti_next"] * 512 > min(u[0] for u in grp):
                    feed(1)
                if len(grp) == 1 and grp[0][1] >= 512:
                    emit_attention(*grp[0])
                else:
                    emit_attention_group(grp)
            while outq:
                outq.pop(0)()
                feed_cb()  # keepalive through the final drain
            feed(n_groups)  # drain any leftovers

    nc.compile()
    return nc


def _prepare(hidden_states, Wqkv_weight, Wqkv_bias, cu_seqlens):
    """Host-side sharding prep. Returns (nc, in_maps, meta)."""
    hs = np.asarray(hidden_states, dtype=np.float32)
    W = np.asarray(Wqkv_weight, dtype=np.float32)
    b = np.asarray(Wqkv_bias, dtype=np.float32).reshape(-1)
    cs = np.asarray(cu_seqlens).astype(np.int64).reshape(-1)
    nnz, dim = hs.shape
    assert dim == DIM and W.shape == (3 * DIM, DIM)
    lengths = tuple(int(cs[i + 1] - cs[i]) for i in range(len(cs) - 1))
    assert sum(lengths) == nnz, (lengths, nnz)

    units, nnz_pad = _padded_units(lengths)
    nc = _build(nnz_pad, units)

    # padded token index map: real token t -> padded column pad_idx[t]
    pad_idx = np.empty(nnz, dtype=np.int64)
    t = 0
    for (O, L) in units:
        pad_idx[t : t + L] = np.arange(O, O + L)
        t += L

    xt_pad = np.zeros((DIM, nnz_pad), dtype=np.float32)
    xt_pad[:, pad_idx] = hs.T
    n_ti = nnz_pad // 512
    xt_np = np.ascontiguousarray(
        xt_pad.reshape(DIM // 128, 128, n_ti, 512).transpose(1, 2, 0, 3)
    ).astype(ml_dtypes.bfloat16)

    in_maps = []
    for c in range(N_CORES):
        r0 = c * HEADS_PER_CORE * HEAD_DIM  # 128c
        Wq = W[r0 : r0 + 128, :]
        Wk = W[DIM + r0 : DIM + r0 + 128, :]
        Wv = W[2 * DIM + r0 : 2 * DIM + r0 + 128, :]
        wqk_np = np.ascontiguousarray(
            np.concatenate([Wq.T, Wk.T], axis=1).reshape(DIM // 128, 128, 256)
        ).astype(ml_dtypes.bfloat16)
        wv_np = np.ascontiguousarray(
            Wv.T.reshape(DIM // 128, 128, 128)
        ).astype(ml_dtypes.bfloat16)
        bias_np = np.ascontiguousarray(
            np.stack([b[r0 : r0 + 128], b[DIM + r0 : DIM + r0 + 128]], axis=1)
        )
        in_maps.append(
            {"xt": xt_np, "wqk": wqk_np, "wv": wv_np, "bias": bias_np}
        )
    meta = {"pad_idx": pad_idx, "nnz": nnz, "bv": b[2 * DIM :]}
    return nc, in_maps, meta


def _postprocess(raws, meta):
    """raws: list of per-core [130, nnz_pad] f32 -> full (nnz, 1024) f32."""
    pad_idx = meta["pad_idx"]
    nnz = meta["nnz"]
    bv = meta["bv"]
    out = np.empty((nnz, DIM), dtype=np.float32)
    for c in range(N_CORES):
        raw = np.asarray(raws[c], dtype=np.float32)
        for h in range(HEADS_PER_CORE):
            num = raw[65 * h : 65 * h + 64][:, pad_idx]  # (64, nnz)
            den = raw[65 * h + 64][pad_idx]  # (nnz,)
            col = c * 128 + h * 64
            out[:, col : col + 64] = num.T / den[:, None] + bv[col : col + 64]
    return out


def kernel(hidden_states, Wqkv_weight, Wqkv_bias, cu_seqlens, max_seqlen=None):
    from concourse.bass_utils import run_bass_kernel_spmd

    nc, in_maps, meta = _prepare(
        hidden_states, Wqkv_weight, Wqkv_bias, cu_seqlens
    )
    res = run_bass_kernel_spmd(nc, in_maps, list(range(N_CORES)))
    return _postprocess(
        [res.results[c]["out"] for c in range(N_CORES)], meta
    )
